# revision 21
# baseline (speedup 1.0000x reference)
"""BiLSTM Trainium2 kernel: B=64, T=512, D=256, H=256, 8 NeuronCores.

Sharding: batch 8-way (8 sequences per core). Each core runs BOTH
directions (forward + backward) as two independent recurrent chains so
the engines can interleave them (one chain's elementwise hides under the
other chain's matmuls).

Host (numpy) does all data movement that is pure layout: per-length
sequence reversal for the backward direction, transposes into the
[feature-on-partition] layouts the device wants, gate reordering
(i,f,g,o) -> (i,f,o,g) so sigmoid/tanh each cover one contiguous column
range, bias folding, and the final gather/mask/concat.

Device (per core):
  phase 1: x projection  xpT[g,(t,b)] = WihT.T @ xT  (+bias, bf16, SBUF-resident)
  phase 2: 512-step recurrence, per step per direction:
      gates.T[128x64] += Whh tiles (16 matmuls, weight-stationary bf16)
      sigmoid/tanh + c/h update on [128,16..64] tiles (DVE+ACT)
  h states staged 32 steps at a time, DMA'd to DRAM as bf16.
"""

import sys

for _p in ("/opt/trn_rl_repo",):
    if _p not in sys.path:
        sys.path.insert(0, _p)

import numpy as np
import ml_dtypes

import concourse.bass as bass
import concourse.mybir as mybir
import concourse.tile as tile
from concourse.tile import add_dep_helper
from concourse import bacc
from concourse.bass_utils import run_bass_kernel_spmd

B, T, D, H = 64, 512, 256, 256
NCORES = 8
BC = B // NCORES          # 8 sequences per core
G4 = 4 * H                # 1024 gate dims
STG = 32                  # recurrence steps per output staging block

BF16 = mybir.dt.bfloat16
F32 = mybir.dt.float32
AF = mybir.ActivationFunctionType

# gate reorder (torch i,f,g,o) -> (i,f,o,g)
_PERM = np.concatenate(
    [np.arange(0, H), np.arange(H, 2 * H), np.arange(3 * H, 4 * H), np.arange(2 * H, 3 * H)]
)


def build_nc(t_steps=T):
    assert t_steps % STG == 0
    nb = t_steps // STG
    TB = t_steps * BC  # (t,b) columns per k-half of xT

    nc = bacc.Bacc(None, target_bir_lowering=False)

    xt_d, wih_d, whh_d, bias_d = {}, {}, {}, {}
    for d in ("f", "b"):
        xt_d[d] = nc.dram_tensor(f"xt_{d}", [128, 2 * TB], BF16, kind="ExternalInput")
        wih_d[d] = nc.dram_tensor(f"wih_{d}", [128, 2048], BF16, kind="ExternalInput")
        whh_d[d] = nc.dram_tensor(f"whh_{d}", [128, 2048], BF16, kind="ExternalInput")
        bias_d[d] = nc.dram_tensor(f"bias_{d}", [128, 8], F32, kind="ExternalInput")
    ident_d = nc.dram_tensor("ident", [128, 128], BF16, kind="ExternalInput")
    out_e = nc.dram_tensor("out", [128, t_steps * 4 * BC], BF16, kind="ExternalOutput")

    with tile.TileContext(nc) as tc:
        with (
            tc.tile_pool(name="big", bufs=1) as big,
            tc.tile_pool(name="work", bufs=3) as work,
            tc.tile_pool(name="stgp", bufs=2) as stgp,
            tc.tile_pool(name="pp", bufs=2, space=bass.MemorySpace.PSUM) as pp,
            tc.tile_pool(name="pr", bufs=3, space=bass.MemorySpace.PSUM) as pr,
        ):
            xt, wih, whh, bias, xpt, cst = {}, {}, {}, {}, {}, {}
            for d in ("f", "b"):
                xt[d] = big.tile([128, 2 * TB], BF16, tag=f"xt{d}", name=f"xt{d}")
                nc.sync.dma_start(xt[d][:], xt_d[d][:])
                wih[d] = big.tile([128, 2048], BF16, tag=f"wih{d}", name=f"wih{d}")
                nc.sync.dma_start(wih[d][:], wih_d[d][:])
                whh[d] = big.tile([128, 2048], BF16, tag=f"whh{d}", name=f"whh{d}")
                nc.sync.dma_start(whh[d][:], whh_d[d][:])
                bias[d] = big.tile([128, 8], F32, tag=f"bias{d}", name=f"bias{d}")
                nc.sync.dma_start(bias[d][:], bias_d[d][:])
                xpt[d] = big.tile([128, t_steps * 8 * BC], BF16, tag=f"xpt{d}", name=f"xpt{d}")
                cst[d] = big.tile([128, 2 * BC], F32, tag=f"c{d}", name=f"c{d}")
                nc.vector.memset(cst[d][:], 0.0)
            zh = big.tile([128, 4 * BC], BF16, tag="zh", name="zh")
            nc.vector.memset(zh[:], 0.0)
            ident = big.tile([128, 128], BF16, tag="ident", name="ident")
            nc.sync.dma_start(ident[:], ident_d[:])

            # ---- phase 1: input projection (emitted lazily, paced into the
            # recurrence loop so it fills engine idle time instead of
            # blocking the first recurrence steps) ----
            # xpt layout per dir: col = j*TB + t*BC + b -> projection writes
            # are contiguous [128,ncols]; the recurrence I-MM reads a strided
            # [128, 8, BC] view.  The bias+copy alternates DVE/ACT so the
            # scheduler-hoisted projection flood saturates neither engine.
            ncols = min(512, TB)
            ntiles = TB // ncols

            proj_groups = [
                (d, nt, j)
                for nt in range(ntiles)
                for d in ("f", "b")
                for j in range(8)
            ]
            proj_n = [0]

            def emit_proj_group(d, nt, j):
                ps = pp.tile([128, ncols], F32, tag="pp", name="pp")
                for kk in (0, 1):
                    nc.tensor.matmul(
                        ps[:],
                        wih[d][:, kk * 1024 + j * 128 : kk * 1024 + (j + 1) * 128],
                        xt[d][:, kk * TB + nt * ncols : kk * TB + (nt + 1) * ncols],
                        start=(kk == 0),
                        stop=(kk == 1),
                    )
                dst = xpt[d][:, j * TB + nt * ncols : j * TB + (nt + 1) * ncols]
                proj_n[0] += 1
                if proj_n[0] % 2 == 0:
                    nc.scalar.activation(
                        dst, ps[:], AF.Identity, bias=bias[d][:, j : j + 1],
                    )
                else:
                    nc.vector.tensor_scalar(
                        dst, ps[:], bias[d][:, j : j + 1], None, mybir.AluOpType.add,
                    )

            # ---- phase 2: recurrence (staggered F/B emission) ----
            # psum(t) = I.T @ xp(t)  (start=True)  then += Whh tiles; the
            # sigmoid reads PSUM directly.  tanh(g) is folded into the wide
            # sigmoid: g rows were pre-scaled x2 on host, tanh(g)=2*sig(2g)-1.
            stg_tiles = {}

            def stg_slot(u):
                return stg_tiles[u // STG], (u % STG) * 4 * BC

            def emit_inject(d, t):
                # xp injection (start=True) emitted ahead of BOTH bursts so it
                # issues in the h-wait idle window, never between the pair.
                ps = pr.tile([128, 8 * BC], F32, tag=f"pr{d}", name=f"pr{d}")
                xv = xpt[d][:].rearrange("p (j tb) -> p j tb", j=8)
                nc.tensor.matmul(
                    ps[:], ident[:], xv[:, :, t * BC : (t + 1) * BC],
                    start=True, stop=False,
                )
                return ps

            def emit_burst(d, doff, t, ps):
                if t == 0:
                    prev = zh[:]
                else:
                    st, off = stg_slot(t - 1)
                    prev = st[:, off : off + 4 * BC]
                for kk in (0, 1):
                    rhs = prev[:, doff + kk * BC : doff + (kk + 1) * BC]
                    for j in range(8):
                        nc.tensor.matmul(
                            ps[:, j * BC : (j + 1) * BC],
                            whh[d][:, kk * 1024 + j * 128 : kk * 1024 + (j + 1) * 128],
                            rhs,
                            start=False,
                            stop=(j == 7 and kk == 1),
                        )
                return ps

            def emit_ew(d, doff, t, ps):
                # state kept halved: s = c/2, so s' = σf*s + (σ(2g)-0.5)*σi
                # and tanh(c) = tanh(2s) via the ACT free input scale.
                st, off = stg_slot(t)
                act = work.tile([128, 8 * BC], F32, tag=f"act{d}", name=f"act{d}")
                nc.scalar.activation(act[:], ps[:], AF.Sigmoid)
                fc = work.tile([128, 2 * BC], F32, tag=f"fc{d}", name=f"fc{d}")
                nc.vector.tensor_mul(fc[:], act[:, 2 * BC : 4 * BC], cst[d][:])
                ht = work.tile([128, 2 * BC], F32, tag=f"ht{d}", name=f"ht{d}")
                nc.vector.scalar_tensor_tensor(
                    ht[:], act[:, 6 * BC : 8 * BC], -0.5, act[:, : 2 * BC],
                    mybir.AluOpType.add, mybir.AluOpType.mult,
                )
                snew = work.tile([128, 2 * BC], F32, tag=f"c{d}", name=f"c{d}", bufs=2)
                nc.vector.tensor_add(snew[:], fc[:], ht[:])
                cst[d] = snew
                th = work.tile([128, 2 * BC], F32, tag=f"th{d}", name=f"th{d}")
                nc.scalar.activation(th[:], snew[:], AF.Tanh, scale=2.0)
                nc.vector.tensor_mul(
                    st[:, off + doff : off + doff + 2 * BC],
                    act[:, 4 * BC : 6 * BC],
                    th[:],
                )

            # upfront: the first two ntiles (steps 0..63); the rest paced via
            # same-engine ordering deps so the scheduler can't hoist them all
            # into the first few (latency-bound) recurrence steps.  One proj
            # matmul per step lands in the post-burst_f tensor idle window;
            # the bias-TS trails 2 steps so it can never head-of-line block
            # the DVE queue (its data is long ready when the FIFO reaches it).
            gq = list(proj_groups)
            n_upfront = min(len(gq), 32)
            for _ in range(n_upfront):
                emit_proj_group(*gq.pop(0))
            n_rest = len(gq)

            ps_f = ps_b = None
            for t in range(t_steps):
                if n_rest:
                    tgt = min(n_rest, (t * n_rest) // max(1, (t_steps - 128)) + 1)
                    while len(gq) > n_rest - tgt:
                        emit_proj_group(*gq.pop(0))
                if t % STG == 0:
                    stg_tiles[t // STG] = stgp.tile(
                        [128, STG * 4 * BC], BF16, tag="stg", name="stg"
                    )
                ps_f = emit_inject("f", t)
                ps_b = emit_inject("b", t)
                emit_burst("f", 0, t, ps_f)
                if t >= 1:
                    emit_ew("b", 2 * BC, t - 1, ps_b_prev)
                    if t % STG == 0:
                        blk = t // STG - 1
                        nc.sync.dma_start(
                            out_e[:, blk * STG * 4 * BC : (blk + 1) * STG * 4 * BC],
                            stg_tiles[blk][:],
                        )
                emit_burst("b", 2 * BC, t, ps_b)
                ps_b_prev = ps_b
                emit_ew("f", 0, t, ps_f)
            emit_ew("b", 2 * BC, t_steps - 1, ps_b_prev)
            blk = nb - 1
            nc.sync.dma_start(
                out_e[:, blk * STG * 4 * BC : (blk + 1) * STG * 4 * BC],
                stg_tiles[blk][:],
            )

    nc.compile()
    return nc


def _prep_core(xs, Wih, Whh, bih, bhh, t_steps):
    """Host-side layout prep for one core, one direction.

    xs: [BC, t, D] f32 (already reversed for the backward direction).
    Returns dict of device arrays.
    """
    TB = t_steps * BC
    Wp = Wih[_PERM].astype(np.float32).copy()   # [1024, 256]
    Wh = Whh[_PERM].astype(np.float32).copy()
    bsum = (bih + bhh)[_PERM].astype(np.float32).copy()
    # tanh(g) is computed as 2*sigmoid(2g)-1 on device: pre-scale g rows x2
    Wp[3 * H :] *= 2.0
    Wh[3 * H :] *= 2.0
    bsum[3 * H :] *= 2.0

    def wt_layout(W):  # [4H, 256] -> [128, 2048] lhsT layout
        WT = W.T.reshape(2, 128, G4).transpose(1, 0, 2).reshape(128, 2 * G4)
        return np.ascontiguousarray(WT).astype(ml_dtypes.bfloat16)

    xT = (
        xs.transpose(2, 1, 0)                   # [256, t, BC]
        .reshape(2, 128, TB)
        .transpose(1, 0, 2)
        .reshape(128, 2 * TB)
    )
    return {
        "xt": np.ascontiguousarray(xT).astype(ml_dtypes.bfloat16),
        "wih": wt_layout(Wp),
        "whh": wt_layout(Wh),
        "bias": np.ascontiguousarray(bsum.reshape(8, 128).T).astype(np.float32),
    }


_NC_CACHE = {}


def _get_nc(t_steps):
    if t_steps not in _NC_CACHE:
        _NC_CACHE[t_steps] = build_nc(t_steps)
    return _NC_CACHE[t_steps]


def kernel(x, input_length, Wih_f, Whh_f, bih_f, bhh_f, Wih_b, Whh_b, bih_b, bhh_b,
           t_steps=T, _want_trace=False):
    x = np.asarray(x, np.float32)
    lens = np.asarray(input_length).astype(np.int64)
    L = t_steps
    tt = np.arange(L)

    nc = _get_nc(t_steps)

    in_maps = []
    for c in range(NCORES):
        bs = slice(c * BC, (c + 1) * BC)
        xs = x[bs, :L]
        ls = lens[bs]
        inv_idx = L - 1 - ((L - ls[:, None] + tt[None, :]) % L)       # [BC, L]
        xn = np.take_along_axis(xs, inv_idx[:, :, None], axis=1)
        pf = _prep_core(xs, Wih_f, Whh_f, bih_f, bhh_f, L)
        pb = _prep_core(xn, Wih_b, Whh_b, bih_b, bhh_b, L)
        in_maps.append(
            {
                "xt_f": pf["xt"], "wih_f": pf["wih"], "whh_f": pf["whh"], "bias_f": pf["bias"],
                "xt_b": pb["xt"], "wih_b": pb["wih"], "whh_b": pb["whh"], "bias_b": pb["bias"],
                "ident": np.eye(128, dtype=np.float32).astype(ml_dtypes.bfloat16),
            }
        )

    kw = {}
    if _want_trace:
        kw = dict(trace=True)
    res = run_bass_kernel_spmd(nc, in_maps, core_ids=list(range(NCORES)), **kw)

    outs = []
    for c in range(NCORES):
        bs = slice(c * BC, (c + 1) * BC)
        ls = lens[bs]
        arr = np.asarray(res.results[c]["out"]).astype(np.float32)
        arr = arr.reshape(128, L, 4, BC)
        fwd = arr[:, :, 0:2, :].transpose(3, 1, 2, 0).reshape(BC, L, 2 * 128)
        bwd = arr[:, :, 2:4, :].transpose(3, 1, 2, 0).reshape(BC, L, 2 * 128)
        bwd_idx = np.clip(ls[:, None] - 1 - tt[None, :], 0, L - 1)
        bwd_g = np.take_along_axis(bwd, bwd_idx[:, :, None], axis=1)
        o = np.concatenate([fwd, bwd_g], axis=-1)
        mask = (tt[None, :] < ls[:, None])[:, :, None]
        outs.append(np.where(mask, o, 0.0).astype(np.float32))
    full = np.concatenate(outs, axis=0)
    if _want_trace:
        return full, res
    return full



# revision 27
# speedup vs baseline: 1.0115x; 1.0115x over previous
"""BiLSTM Trainium2 kernel: B=64, T=512, D=256, H=256, 8 NeuronCores.

Sharding: batch 8-way (8 sequences per core). Each core runs BOTH
directions (forward + backward) as two independent recurrent chains so
the engines can interleave them (one chain's elementwise hides under the
other chain's matmuls).

Host (numpy) does all data movement that is pure layout: per-length
sequence reversal for the backward direction, transposes into the
[feature-on-partition] layouts the device wants, gate reordering
(i,f,g,o) -> (i,f,o,g) so sigmoid/tanh each cover one contiguous column
range, bias folding, and the final gather/mask/concat.

Device (per core):
  phase 1: x projection  xpT[g,(t,b)] = WihT.T @ xT  (+bias, bf16, SBUF-resident)
  phase 2: 512-step recurrence, per step per direction:
      gates.T[128x64] += Whh tiles (16 matmuls, weight-stationary bf16)
      sigmoid/tanh + c/h update on [128,16..64] tiles (DVE+ACT)
  h states staged 32 steps at a time, DMA'd to DRAM as bf16.
"""

import sys

for _p in ("/opt/trn_rl_repo",):
    if _p not in sys.path:
        sys.path.insert(0, _p)

import numpy as np
import ml_dtypes

import concourse.bass as bass
import concourse.mybir as mybir
import concourse.tile as tile
from concourse.tile import add_dep_helper
from concourse import bacc
from concourse.bass_utils import run_bass_kernel_spmd

B, T, D, H = 64, 512, 256, 256
NCORES = 8
BC = B // NCORES          # 8 sequences per core
G4 = 4 * H                # 1024 gate dims
STG = 32                  # recurrence steps per output staging block

BF16 = mybir.dt.bfloat16
F32 = mybir.dt.float32
AF = mybir.ActivationFunctionType

# gate reorder (torch i,f,g,o) -> (i,f,o,g)
_PERM = np.concatenate(
    [np.arange(0, H), np.arange(H, 2 * H), np.arange(3 * H, 4 * H), np.arange(2 * H, 3 * H)]
)


def build_nc(t_steps=T):
    nb = (t_steps + STG - 1) // STG  # last block may be partial
    TB = t_steps * BC  # (t,b) columns per k-half of xT

    nc = bacc.Bacc(None, target_bir_lowering=False)

    xt_d, wih_d, whh_d, bias_d = {}, {}, {}, {}
    for d in ("f", "b"):
        xt_d[d] = nc.dram_tensor(f"xt_{d}", [128, 2 * TB], BF16, kind="ExternalInput")
        wih_d[d] = nc.dram_tensor(f"wih_{d}", [128, 2048], BF16, kind="ExternalInput")
        whh_d[d] = nc.dram_tensor(f"whh_{d}", [128, 2048], BF16, kind="ExternalInput")
        bias_d[d] = nc.dram_tensor(f"bias_{d}", [128, 8], F32, kind="ExternalInput")
    ident_d = nc.dram_tensor("ident", [128, 128], BF16, kind="ExternalInput")
    out_e = nc.dram_tensor("out", [128, nb * STG * 4 * BC], BF16, kind="ExternalOutput")

    with tile.TileContext(nc) as tc:
        with (
            tc.tile_pool(name="big", bufs=1) as big,
            tc.tile_pool(name="work", bufs=3) as work,
            tc.tile_pool(name="stgp", bufs=2) as stgp,
            tc.tile_pool(name="pp", bufs=2, space=bass.MemorySpace.PSUM) as pp,
            tc.tile_pool(name="pr", bufs=3, space=bass.MemorySpace.PSUM) as pr,
        ):
            xt, wih, whh, bias, xpt, cst = {}, {}, {}, {}, {}, {}
            for d in ("f", "b"):
                xt[d] = big.tile([128, 2 * TB], BF16, tag=f"xt{d}", name=f"xt{d}")
                nc.sync.dma_start(xt[d][:], xt_d[d][:])
                wih[d] = big.tile([128, 2048], BF16, tag=f"wih{d}", name=f"wih{d}")
                nc.sync.dma_start(wih[d][:], wih_d[d][:])
                whh[d] = big.tile([128, 2048], BF16, tag=f"whh{d}", name=f"whh{d}")
                nc.sync.dma_start(whh[d][:], whh_d[d][:])
                bias[d] = big.tile([128, 8], F32, tag=f"bias{d}", name=f"bias{d}")
                nc.sync.dma_start(bias[d][:], bias_d[d][:])
                xpt[d] = big.tile([128, t_steps * 8 * BC], BF16, tag=f"xpt{d}", name=f"xpt{d}")
                cst[d] = big.tile([128, 2 * BC], F32, tag=f"c{d}", name=f"c{d}")
                nc.vector.memset(cst[d][:], 0.0)
            zh = big.tile([128, 4 * BC], BF16, tag="zh", name="zh")
            nc.vector.memset(zh[:], 0.0)
            ident = big.tile([128, 128], BF16, tag="ident", name="ident")
            nc.sync.dma_start(ident[:], ident_d[:])

            # ---- phase 1: input projection (emitted lazily, paced into the
            # recurrence loop so it fills engine idle time instead of
            # blocking the first recurrence steps) ----
            # xpt layout per dir: col = j*TB + t*BC + b -> projection writes
            # are contiguous [128,ncols]; the recurrence I-MM reads a strided
            # [128, 8, BC] view.  The bias+copy alternates DVE/ACT so the
            # scheduler-hoisted projection flood saturates neither engine.
            ncols = TB // 8  # = t_steps cols per tile (8 ntiles), divides TB
            ntiles = TB // ncols

            proj_groups = [
                (d, nt, j)
                for nt in range(ntiles)
                for d in ("f", "b")
                for j in range(8)
            ]
            proj_n = [0]

            def emit_proj_group(d, nt, j):
                ps = pp.tile([128, ncols], F32, tag="pp", name="pp")
                for kk in (0, 1):
                    nc.tensor.matmul(
                        ps[:],
                        wih[d][:, kk * 1024 + j * 128 : kk * 1024 + (j + 1) * 128],
                        xt[d][:, kk * TB + nt * ncols : kk * TB + (nt + 1) * ncols],
                        start=(kk == 0),
                        stop=(kk == 1),
                    )
                dst = xpt[d][:, j * TB + nt * ncols : j * TB + (nt + 1) * ncols]
                proj_n[0] += 1
                if proj_n[0] % 2 == 0:
                    nc.scalar.activation(
                        dst, ps[:], AF.Identity, bias=bias[d][:, j : j + 1],
                    )
                else:
                    nc.vector.tensor_scalar(
                        dst, ps[:], bias[d][:, j : j + 1], None, mybir.AluOpType.add,
                    )

            # ---- phase 2: recurrence (staggered F/B emission) ----
            # psum(t) = I.T @ xp(t)  (start=True)  then += Whh tiles; the
            # sigmoid reads PSUM directly.  tanh(g) is folded into the wide
            # sigmoid: g rows were pre-scaled x2 on host, tanh(g)=2*sig(2g)-1.
            stg_tiles = {}

            def stg_slot(u):
                return stg_tiles[u // STG], (u % STG) * 4 * BC

            def emit_inject(d, t):
                # xp injection (start=True) emitted ahead of BOTH bursts so it
                # issues in the h-wait idle window, never between the pair.
                ps = pr.tile([128, 8 * BC], F32, tag=f"pr{d}", name=f"pr{d}")
                xv = xpt[d][:].rearrange("p (j tb) -> p j tb", j=8)
                nc.tensor.matmul(
                    ps[:], ident[:], xv[:, :, t * BC : (t + 1) * BC],
                    start=True, stop=False,
                )
                return ps

            def emit_burst(d, doff, t, ps):
                if t == 0:
                    prev = zh[:]
                else:
                    st, off = stg_slot(t - 1)
                    prev = st[:, off : off + 4 * BC]
                for kk in (0, 1):
                    rhs = prev[:, doff + kk * BC : doff + (kk + 1) * BC]
                    for j in range(8):
                        nc.tensor.matmul(
                            ps[:, j * BC : (j + 1) * BC],
                            whh[d][:, kk * 1024 + j * 128 : kk * 1024 + (j + 1) * 128],
                            rhs,
                            start=False,
                            stop=(j == 7 and kk == 1),
                        )
                return ps

            def emit_ew(d, doff, t, ps):
                # state kept halved: s = c/2, so s' = σf*s + (σ(2g)-0.5)*σi
                # and tanh(c) = tanh(2s) via the ACT free input scale.
                st, off = stg_slot(t)
                act = work.tile([128, 8 * BC], F32, tag=f"act{d}", name=f"act{d}")
                nc.scalar.activation(act[:], ps[:], AF.Sigmoid)
                fc = work.tile([128, 2 * BC], F32, tag=f"fc{d}", name=f"fc{d}")
                nc.vector.tensor_mul(fc[:], act[:, 2 * BC : 4 * BC], cst[d][:])
                ht = work.tile([128, 2 * BC], F32, tag=f"ht{d}", name=f"ht{d}")
                nc.vector.scalar_tensor_tensor(
                    ht[:], act[:, 6 * BC : 8 * BC], -0.5, act[:, : 2 * BC],
                    mybir.AluOpType.add, mybir.AluOpType.mult,
                )
                snew = work.tile([128, 2 * BC], F32, tag=f"c{d}", name=f"c{d}", bufs=2)
                nc.vector.tensor_add(snew[:], fc[:], ht[:])
                cst[d] = snew
                th = work.tile([128, 2 * BC], F32, tag=f"th{d}", name=f"th{d}")
                nc.scalar.activation(th[:], snew[:], AF.Tanh, scale=2.0)
                nc.vector.tensor_mul(
                    st[:, off + doff : off + doff + BC],
                    act[:, 4 * BC : 5 * BC],
                    th[:, :BC],
                )
                nc.vector.tensor_mul(
                    st[:, off + doff + BC : off + doff + 2 * BC],
                    act[:, 5 * BC : 6 * BC],
                    th[:, BC:],
                )

            # upfront: the first two ntiles (steps 0..63); the rest paced via
            # same-engine ordering deps so the scheduler can't hoist them all
            # into the first few (latency-bound) recurrence steps.  One proj
            # matmul per step lands in the post-burst_f tensor idle window;
            # the bias-TS trails 2 steps so it can never head-of-line block
            # the DVE queue (its data is long ready when the FIFO reaches it).
            gq = list(proj_groups)
            n_upfront = min(len(gq), 32)
            for _ in range(n_upfront):
                emit_proj_group(*gq.pop(0))
            n_rest = len(gq)

            ps_f = ps_b = None
            for t in range(t_steps):
                if n_rest:
                    tgt = min(n_rest, (t * n_rest) // max(1, (t_steps - 128)) + 1)
                    while len(gq) > n_rest - tgt:
                        emit_proj_group(*gq.pop(0))
                if t % STG == 0:
                    stg_tiles[t // STG] = stgp.tile(
                        [128, STG * 4 * BC], BF16, tag="stg", name="stg"
                    )
                ps_f = emit_inject("f", t)
                ps_b = emit_inject("b", t)
                emit_burst("f", 0, t, ps_f)
                if t >= 1:
                    emit_ew("b", 2 * BC, t - 1, ps_b_prev)
                    if t % STG == 0:
                        blk = t // STG - 1
                        nc.sync.dma_start(
                            out_e[:, blk * STG * 4 * BC : (blk + 1) * STG * 4 * BC],
                            stg_tiles[blk][:],
                        )
                emit_burst("b", 2 * BC, t, ps_b)
                ps_b_prev = ps_b
                emit_ew("f", 0, t, ps_f)
            emit_ew("b", 2 * BC, t_steps - 1, ps_b_prev)
            blk = nb - 1
            nc.sync.dma_start(
                out_e[:, blk * STG * 4 * BC : (blk + 1) * STG * 4 * BC],
                stg_tiles[blk][:],
            )

    nc.compile()
    return nc


def _prep_core(xs, Wih, Whh, bih, bhh, t_steps):
    """Host-side layout prep for one core, one direction.

    xs: [BC, t, D] f32 (already reversed for the backward direction).
    Returns dict of device arrays.
    """
    TB = t_steps * BC
    Wp = Wih[_PERM].astype(np.float32).copy()   # [1024, 256]
    Wh = Whh[_PERM].astype(np.float32).copy()
    bsum = (bih + bhh)[_PERM].astype(np.float32).copy()
    # tanh(g) is computed as 2*sigmoid(2g)-1 on device: pre-scale g rows x2
    Wp[3 * H :] *= 2.0
    Wh[3 * H :] *= 2.0
    bsum[3 * H :] *= 2.0

    def wt_layout(W):  # [4H, 256] -> [128, 2048] lhsT layout
        WT = W.T.reshape(2, 128, G4).transpose(1, 0, 2).reshape(128, 2 * G4)
        return np.ascontiguousarray(WT).astype(ml_dtypes.bfloat16)

    xT = (
        xs.transpose(2, 1, 0)                   # [256, t, BC]
        .reshape(2, 128, TB)
        .transpose(1, 0, 2)
        .reshape(128, 2 * TB)
    )
    return {
        "xt": np.ascontiguousarray(xT).astype(ml_dtypes.bfloat16),
        "wih": wt_layout(Wp),
        "whh": wt_layout(Wh),
        "bias": np.ascontiguousarray(bsum.reshape(8, 128).T).astype(np.float32),
    }


_NC_CACHE = {}


def _get_nc(t_steps):
    if t_steps not in _NC_CACHE:
        _NC_CACHE[t_steps] = build_nc(t_steps)
    return _NC_CACHE[t_steps]


def kernel(x, input_length, Wih_f, Whh_f, bih_f, bhh_f, Wih_b, Whh_b, bih_b, bhh_b,
           t_steps=T, _want_trace=False):
    x = np.asarray(x, np.float32)
    lens = np.asarray(input_length).astype(np.int64)
    L = t_steps
    # Device runs only max(len) steps: every output at t >= len is masked to
    # zero, and both scans' valid regions live in t < len <= TD.
    TD = max(STG, min(L, int(lens.max())))
    tt = np.arange(TD)

    nc = _get_nc(TD)

    in_maps = []
    for c in range(NCORES):
        bs = slice(c * BC, (c + 1) * BC)
        xs = x[bs, :TD]
        ls = lens[bs]
        inv_idx = TD - 1 - ((TD - ls[:, None] + tt[None, :]) % TD)    # [BC, TD]
        xn = np.take_along_axis(xs, inv_idx[:, :, None], axis=1)
        pf = _prep_core(xs, Wih_f, Whh_f, bih_f, bhh_f, TD)
        pb = _prep_core(xn, Wih_b, Whh_b, bih_b, bhh_b, TD)
        in_maps.append(
            {
                "xt_f": pf["xt"], "wih_f": pf["wih"], "whh_f": pf["whh"], "bias_f": pf["bias"],
                "xt_b": pb["xt"], "wih_b": pb["wih"], "whh_b": pb["whh"], "bias_b": pb["bias"],
                "ident": np.eye(128, dtype=np.float32).astype(ml_dtypes.bfloat16),
            }
        )

    kw = {}
    if _want_trace:
        kw = dict(trace=True)
    res = run_bass_kernel_spmd(nc, in_maps, core_ids=list(range(NCORES)), **kw)

    TDp = (TD + STG - 1) // STG * STG  # device output is padded to full blocks
    outs = []
    for c in range(NCORES):
        bs = slice(c * BC, (c + 1) * BC)
        ls = lens[bs]
        arr = np.asarray(res.results[c]["out"]).astype(np.float32)
        arr = arr.reshape(128, TDp, 4, BC)[:, :TD]
        fwd = arr[:, :, 0:2, :].transpose(3, 1, 2, 0).reshape(BC, TD, 2 * 128)
        bwd = arr[:, :, 2:4, :].transpose(3, 1, 2, 0).reshape(BC, TD, 2 * 128)
        bwd_idx = np.clip(ls[:, None] - 1 - tt[None, :], 0, TD - 1)
        bwd_g = np.take_along_axis(bwd, bwd_idx[:, :, None], axis=1)
        o = np.concatenate([fwd, bwd_g], axis=-1)
        mask = (tt[None, :] < ls[:, None])[:, :, None]
        o = np.where(mask, o, 0.0).astype(np.float32)
        if TD < L:
            o = np.pad(o, ((0, 0), (0, L - TD), (0, 0)))
        outs.append(o)
    full = np.concatenate(outs, axis=0)
    if _want_trace:
        return full, res
    return full

